# revision 1
# baseline (speedup 1.0000x reference)
"""Trainium2 Bass kernel for a 2-layer LSTM encoder (relu cell activation).

Problem: x[128, 512, 64] -> LSTM(256, relu, seq) -> LSTM(128, relu, last) -> out[128, 128]

Strategy (per core, data-parallel over batch, 16 rows/core):
  - "units-on-partition" transposed layout: all recurrent state kept as
    [units(partition), batch(free)] so the sequential recurrence needs no
    per-step transposes and elementwise ops use all 128 lanes.
  - Bulk (time-parallel) matmuls for x@W1+b1 and h1@W2+b2, computed per
    32-step chunk with amortized weight loads; results stored bf16 in SBUF
    and injected into each step's PSUM accumulator via an identity matmul
    (start=True), after which the recurrent h@U matmuls accumulate on top.
  - One fused PSUM tile [128, 4(gates I,F,O,G), 48] per step holds layer-1
    (2 unit-chunks of 128) and layer-2 (lagged by one chunk) gates, so
    sigmoid / relu / cell-update each take ONE instruction for both layers.
  - bf16 weights + h (FWL halves LDWEIGHTS cost), fp32 PSUM + cell state.

Layer-2 recurrence lags layer-1 by one chunk (CH steps) so its bulk input
matmul (h1 @ W2) can be computed chunk-wise.
"""

import numpy as np
from contextlib import ExitStack

import concourse.bass as bass
import concourse.tile as tile
from concourse import bacc
from concourse import mybir
from concourse.bass_utils import run_bass_kernel_spmd

fp32 = mybir.dt.float32
bf16 = mybir.dt.bfloat16
AF = mybir.ActivationFunctionType

B, T, F = 128, 512, 64
U1, U2 = 256, 128
NCORES = 8
BL = B // NCORES  # 16 batch rows per core

# gate block order in the fused z tile: I, F, O, G (relu'd gate last)
# Keras weight column order is i, f, g, o -> column offsets per block:
COLMAP1 = [0 * U1, 1 * U1, 3 * U1, 2 * U1]  # into [*, 4*U1]
COLMAP2 = [0 * U2, 1 * U2, 3 * U2, 2 * U2]  # into [*, 4*U2]

# packed constant blocks (single DMA each to limit sync-wait fan-in)
CB16_COLS = 8 * U1 + 12 * U2 + 128   # u1(2x1024) | w2(2x512) | u2(512) | eye(128)
CF32_COLS = 4 * U1 + 128 + 8 + 4     # w1(1024, rows0:64) | eye(128) | b1p(8) | b2p(4)


def build(T_=T, CH=32, nonzero_bias=False, reps=1):
    """Build the per-core Bass program. Returns nc.

    reps>1 repeats the whole computation (for timing-by-differencing)."""
    assert T_ % CH == 0 and (CH * BL) % 128 == 0
    NCH = T_ // CH
    NJ = CH * BL // 128  # 128-row blocks per x chunk
    RING = 3 * CH  # h ring slots (multiple of CH, covers lag + slack)
    LAG = CH       # L2 lags L1 by one chunk
    TOT = T_ + LAG

    nc = bacc.Bacc("TRN2", target_bir_lowering=False, debug=False)

    x_d = nc.declare_dram_parameter("x", [T_ * BL, F], fp32, isOutput=False)
    cb_d = nc.declare_dram_parameter("cb16", [128, CB16_COLS], bf16, isOutput=False)
    cf_d = nc.declare_dram_parameter("cf32", [128, CF32_COLS], fp32, isOutput=False)
    out_d = nc.declare_dram_parameter("out", [BL, U2], fp32, isOutput=True)

    with tile.TileContext(nc) as tc, ExitStack() as ctx:
        const_p = ctx.enter_context(tc.tile_pool(name="const", bufs=1))
        xst_p = ctx.enter_context(tc.tile_pool(name="xst", bufs=2))
        xT_p = ctx.enter_context(tc.tile_pool(name="xT", bufs=2))
        zx_p = ctx.enter_context(tc.tile_pool(name="zx", bufs=3))
        ew_p = ctx.enter_context(tc.tile_pool(name="ew", bufs=3))
        state_p = ctx.enter_context(tc.tile_pool(name="state", bufs=1))
        pz_p = ctx.enter_context(tc.tile_pool(name="pz", bufs=2, space="PSUM"))
        pzg_p = ctx.enter_context(tc.tile_pool(name="pzg", bufs=2, space="PSUM"))
        pb1_p = ctx.enter_context(tc.tile_pool(name="pb1", bufs=2, space="PSUM"))
        pb2_p = ctx.enter_context(tc.tile_pool(name="pb2", bufs=2, space="PSUM"))

        # ---- load all constants with TWO DMAs (avoids sync-wait fan-in) ----
        cb = const_p.tile([128, CB16_COLS], bf16, name="cb")
        nc.sync.dma_start(cb[:, :], cb_d[:, :])
        cf = const_p.tile([128, CF32_COLS], fp32, name="cf")
        nc.sync.dma_start(cf[:, :], cf_d[:, :])
        # bf16 views
        u1sb = [cb[:, 0:4 * U1], cb[:, 4 * U1:8 * U1]]
        w2sb = [cb[:, 8 * U1:8 * U1 + 4 * U2],
                cb[:, 8 * U1 + 4 * U2:8 * U1 + 8 * U2]]
        u2sb = cb[0:U2, 8 * U1 + 8 * U2:8 * U1 + 12 * U2]
        idb = cb[:, 8 * U1 + 12 * U2:8 * U1 + 12 * U2 + 128]
        # fp32 views
        w1sb = cf[0:F, 0:4 * U1]
        idf = cf[:, 4 * U1:4 * U1 + 128]
        b1sb = cf[:, 4 * U1 + 128:4 * U1 + 136]
        b2sb = cf[:, 4 * U1 + 136:4 * U1 + 140]

        # ---- persistent state ----
        # gc: [128, (relu_g | c)] x 48; keeping g and c adjacent lets one DVE
        # multiply compute both i*g and f*c
        gc_sb = state_p.tile([128, 96], fp32)
        c_sb = gc_sb[:, 48:96]
        # h ring: slot t%RING -> [128, (h1_uc0|h1_uc1|h2), 16b] bf16
        h_ring = state_p.tile([128, RING, 48], bf16)

        # fence the preamble (const DMA) so later instructions sync through
        # one barrier instead of fanning in on many queues
        tc.strict_bb_all_engine_barrier()

        # per-chunk zx buffers: chunk k tile holds L1 x-part for chunk k and
        # L2 x-part (h1@W2) for chunk k-1, interleaved [bi][uc0|uc1|L2]x16b,
        # so each step needs ONE inject matmul.
        zx1_tiles = [None] * (NCH + 1)

        def _get_zx(k):
            if zx1_tiles[k] is None:
                zx1 = zx_p.tile([128, CH, 192], bf16, name="zx1", tag="zx1")
                zx1_tiles[k] = zx1
                if k == 0 or k >= NCH:
                    # unwritten columns are injected before being overwritten;
                    # clear once so no stray NaN bit patterns enter PSUM
                    nc.vector.memset(zx1[:, :, :], 0.0)
            return zx1_tiles[k]

        def bulk_l1x(k):
            """x chunk k -> transpose -> z1x = W1.T @ xT (+b1) -> zx1[k] bf16."""
            xst = xst_p.tile([128, NJ, F], fp32, name="xst")
            nc.sync.dma_start(
                xst[:, :, :],
                x_d.rearrange("(c j p) f -> c p j f", j=NJ, p=128)[k],
            )
            xTc = xT_p.tile([F, CH * BL], fp32, name="xTc")
            for j in range(NJ):
                ptx = pb1_p.tile([F, 128], fp32, name="ptx", tag="pb1")
                nc.tensor.transpose(ptx[:, :], xst[:, j, :], idf[:, :])
                nc.vector.tensor_copy(xTc[:, j * 128:(j + 1) * 128], ptx[:, :])
            zx1 = _get_zx(k)
            NSUB = (CH * BL + 511) // 512  # keep each bulk matmul to one bank
            TSUB = CH // NSUB
            for bi in range(4):
                for uc in range(2):
                    for sj in range(NSUB):
                        pb = pb1_p.tile([128, TSUB * BL], fp32, name="pb", tag="pb1")
                        nc.tensor.matmul(
                            pb[:, :],
                            w1sb[:, COLMAP1[bi] + uc * 128:COLMAP1[bi] + (uc + 1) * 128],
                            xTc[:, sj * TSUB * BL:(sj + 1) * TSUB * BL],
                            start=True, stop=True,
                        )
                        src = pb.rearrange("p (t b) -> p t b", b=BL)
                        dst = zx1[:, sj * TSUB:(sj + 1) * TSUB,
                                  bi * 48 + uc * 16:bi * 48 + (uc + 1) * 16]
                        if nonzero_bias:
                            nc.vector.tensor_scalar_add(
                                dst, src, b1sb[:, bi * 2 + uc:bi * 2 + uc + 1])
                        else:
                            nc.scalar.copy(dst, src)

        def bulk_l2x(k):
            """z2x = W2.T @ h1[chunk k] (+b2) -> zx tile k+1, L2 columns."""
            zx2 = _get_zx(k + 1)
            NSUB = (CH * BL + 511) // 512
            TSUB = CH // NSUB
            for bi in range(4):
                for sj in range(NSUB):
                    rs = (k * CH + sj * TSUB) % RING
                    pb = pb2_p.tile([128, TSUB * BL], fp32, name="pb2t", tag="pb2")
                    for kc in range(2):
                        nc.tensor.matmul(
                            pb[:, :],
                            w2sb[kc][:, COLMAP2[bi]:COLMAP2[bi] + 128],
                            h_ring[:, rs:rs + TSUB, kc * 16:(kc + 1) * 16],
                            start=(kc == 0), stop=(kc == 1),
                        )
                    src = pb.rearrange("p (t b) -> p t b", b=BL)
                    dst = zx2[:, sj * TSUB:(sj + 1) * TSUB, bi * 48 + 32:bi * 48 + 48]
                    if nonzero_bias:
                        nc.vector.tensor_scalar_add(dst, src, b2sb[:, bi:bi + 1])
                    else:
                        nc.scalar.copy(dst, src)

        def emit_body():
            nonlocal h2f
            zx1_tiles[:] = [None] * (NCH + 1)
            bulk_l1x(0)
            for t in range(TOT):
                k, tl = t // CH, t % CH
                s = t - LAG  # layer-2 step
                if t < T_ and tl == 0 and k + 1 < NCH:
                    bulk_l1x(k + 1)
                if t >= CH and (t - CH) % CH == 0 and (t - CH) // CH < NCH:
                    bulk_l2x((t - CH) // CH)

                # fused z psum: I,F,O blocks in one bank; G in its OWN bank so
                # DVE's relu(g) and ACT's sigmoid read different banks (parallel)
                pz = pz_p.tile([128, 3, 48], fp32, name="pz")
                pzg = pzg_p.tile([128, 48], fp32, name="pzg")
                zxt = zx1_tiles[k]
                hp = h_ring[:, (t - 1) % RING, :]
                # G-bank group first: relu(g) unblocks a few MMs in
                gms = [(pzg[:, :], idb[:, :], zxt[:, tl, 144:192])]
                ims = [(pz[:, :, :], idb[:, :], zxt[:, tl, 0:144])]
                for bi in range(4):
                    dst_list = gms if bi == 3 else ims
                    if t < T_:
                        for uc in range(2):
                            cc = COLMAP1[bi] + uc * 128
                            for kc in range(2):
                                o_ap = (pzg[:, uc * 16:(uc + 1) * 16] if bi == 3
                                        else pz[:, bi, uc * 16:(uc + 1) * 16])
                                dst_list.append((o_ap, u1sb[kc][:, cc:cc + 128],
                                                 hp[:, kc * 16:(kc + 1) * 16]))
                    if s >= 0:
                        o_ap = (pzg[:, 32:48] if bi == 3 else pz[:, bi, 32:48])
                        dst_list.append((o_ap,
                                         u2sb[:, COLMAP2[bi]:COLMAP2[bi] + 128],
                                         hp[:, 32:48]))
                for i, (o, l, r) in enumerate(gms):
                    nc.tensor.matmul(o, l, r, start=(i == 0), stop=(i == len(gms) - 1))
                for i, (o, l, r) in enumerate(ims):
                    nc.tensor.matmul(o, l, r, start=(i == 0), stop=(i == len(ims) - 1))

                # elementwise: relu(g) (DVE, G bank) runs parallel to
                # sigmoid(i,f) (ACT, IFO bank); sigmoid(o) overlaps cell ops
                gates = ew_p.tile([128, 3, 48], fp32, name="gates")
                nc.vector.tensor_scalar_max(gc_sb[:, 0:48], pzg[:, :], 0.0)
                nc.scalar.activation(gates[:, 0:2, :], pz[:, 0:2, :], AF.Sigmoid)
                nc.scalar.activation(gates[:, 2, :], pz[:, 2, :], AF.Sigmoid)
                # one multiply computes [i*g | f*c]
                igfc = ew_p.tile([128, 96], fp32, name="igfc")
                nc.vector.tensor_mul(igfc[:, :], gates[:, 0:2, :], gc_sb[:, :])
                nc.vector.tensor_add(c_sb, igfc[:, 0:48], igfc[:, 48:96])
                rc = ew_p.tile([128, 48], fp32, name="rc")
                nc.vector.tensor_scalar_max(rc[:, :], c_sb, 0.0)
                slot = t % RING
                nc.vector.tensor_mul(h_ring[:, slot, :], gates[:, 2, :], rc[:, :])

                if t == LAG - 1:
                    # reset L2 state before its first real step
                    nc.vector.memset(h_ring[:, slot, 32:48], 0.0)
                    nc.vector.memset(gc_sb[:, 80:96], 0.0)
                if t == TOT - 1:
                    h2f = ew_p.tile([128, BL], fp32, name="h2f")
                    nc.vector.tensor_mul(h2f[:, :], gates[:, 2, 32:48], rc[:, 32:48])


        h2f = None
        for _rep in range(reps):
            nc.vector.memset(gc_sb[:, :], 0.0)
            nc.vector.memset(h_ring[:, RING - 1, :], 0.0)  # h(-1) = 0
            emit_body()

        # final: h2 [128u, 16b] -> out [16b, 128u]
        pfin = pb1_p.tile([BL, 128], fp32, name="pfin", tag="pb1")
        nc.tensor.transpose(pfin[:, :], h2f[:, :], idf[:, :])
        osb = ew_p.tile([BL, 128], fp32, name="osb")
        nc.scalar.copy(osb[:, :], pfin[:, :])
        nc.sync.dma_start(out_d[:, :], osb[:, :])

    nc.finalize()
    return nc


_cache = {}


def _get_nc(T_=T, CH=32, nonzero_bias=False, reps=1):
    key = (T_, CH, nonzero_bias, reps)
    if key not in _cache:
        _cache[key] = build(T_, CH, nonzero_bias, reps)
    return _cache[key]


def make_inputs(x, W1, U1w, b1, W2, U2w, b2, T_=T):
    """Host-side packing -> per-core input maps."""
    bf = mybir.dt.np(bf16)
    x = np.asarray(x, np.float32)
    u1b = np.asarray(U1w, np.float32).astype(bf)
    u2b = np.asarray(U2w, np.float32).astype(bf)
    w2b = np.asarray(W2, np.float32).astype(bf)
    b1 = np.asarray(b1, np.float32)
    b2 = np.asarray(b2, np.float32)
    b1p = np.zeros((128, 8), np.float32)
    for bi in range(4):
        for uc in range(2):
            b1p[:, bi * 2 + uc] = b1[COLMAP1[bi] + uc * 128:COLMAP1[bi] + (uc + 1) * 128]
    b2p = np.zeros((128, 4), np.float32)
    for bi in range(4):
        b2p[:, bi] = b2[COLMAP2[bi]:COLMAP2[bi] + 128]
    cb16 = np.zeros((128, CB16_COLS), bf)
    cb16[:, 0:1024] = u1b[0:128]
    cb16[:, 1024:2048] = u1b[128:256]
    cb16[:, 2048:2560] = w2b[0:128]
    cb16[:, 2560:3072] = w2b[128:256]
    cb16[:, 3072:3584] = u2b
    cb16[:, 3584:3712] = np.eye(128).astype(bf)
    cf32 = np.zeros((128, CF32_COLS), np.float32)
    cf32[0:64, 0:1024] = np.asarray(W1, np.float32)
    cf32[:, 1024:1152] = np.eye(128, dtype=np.float32)
    cf32[:, 1152:1160] = b1p
    cf32[:, 1160:1164] = b2p
    common = dict(cb16=cb16, cf32=cf32)
    xr = x.reshape(NCORES, BL, x.shape[1], F)
    in_maps = []
    for c in range(NCORES):
        xc = np.ascontiguousarray(
            np.swapaxes(xr[c], 0, 1).reshape(x.shape[1] * BL, F))
        m = dict(common)
        m["x"] = xc[:T_ * BL]
        in_maps.append(m)
    nonzero_bias = bool(np.any(b1) or np.any(b2))
    return in_maps, nonzero_bias


def run(inputs, T_=T, CH=32, trace=False, reps=1):
    """inputs: dict from reference.setup_inputs(). Returns (out, exec_time_ns)."""
    in_maps, nzb = make_inputs(
        inputs["x"], inputs["W1"], inputs["U1"], inputs["b1"],
        inputs["W2"], inputs["U2"], inputs["b2"], T_=T_)
    nc = _get_nc(T_, CH, nzb, reps)
    res = run_bass_kernel_spmd(nc, in_maps, list(range(NCORES)), trace=trace)
    out = np.concatenate(
        [res.results[c]["out"] for c in range(NCORES)], axis=0)
    return np.ascontiguousarray(out, dtype=np.float32), res.exec_time_ns


def build_null(T_=T):
    """Same I/O signature, trivial compute — for wall-clock overhead calibration."""
    nc = bacc.Bacc("TRN2", target_bir_lowering=False, debug=False)
    nc.declare_dram_parameter("x", [T_ * BL, F], fp32, isOutput=False)
    nc.declare_dram_parameter("cb16", [128, CB16_COLS], bf16, isOutput=False)
    cf_d = nc.declare_dram_parameter("cf32", [128, CF32_COLS], fp32, isOutput=False)
    out_d = nc.declare_dram_parameter("out", [BL, U2], fp32, isOutput=True)
    with tile.TileContext(nc) as tc, ExitStack() as ctx:
        p = ctx.enter_context(tc.tile_pool(name="p", bufs=1))
        t0 = p.tile([BL, 128], fp32)
        nc.sync.dma_start(t0[:, :], cf_d[0:BL, 1024:1152])
        nc.sync.dma_start(out_d[:, :], t0[:, :])
    nc.finalize()
    return nc


def run_null(inputs, T_=T):
    key = ("null", T_)
    if key not in _cache:
        _cache[key] = build_null(T_)
    in_maps, _ = make_inputs(
        inputs["x"], inputs["W1"], inputs["U1"], inputs["b1"],
        inputs["W2"], inputs["U2"], inputs["b2"], T_=T_)
    res = run_bass_kernel_spmd(_cache[key], in_maps, list(range(NCORES)))
    return res


def kernel(x, W1, U1, b1, W2, U2, b2):
    out, _ = run(dict(x=x, W1=W1, U1=U1, b1=b1, W2=W2, U2=U2, b2=b2))
    return out

